# revision 10
# baseline (speedup 1.0000x reference)
"""Trainium2 Bass kernel for BaselineKNNModel (cosine-sim KNN classifier).

Contract: kernel(**inputs) takes FULL inputs (x [2048,512] f32,
embeddings [100000,512] f32, labels [100000] int) and returns the FULL
output (pred [2048] labels.dtype), distributing work across 8 NeuronCores.

Strategy (database-parallel, per sharding hint):
 - Host: normalize embeddings (cosine denominator), pad N 100000->102400,
   transpose to [512, N]; shard along N across 8 cores (12800 each).
   x normalization is skipped: per-query positive scaling cannot change
   that query's top-k ranking.
 - Device (SPMD, per core): sim tile [128 q, 512 c] = xT.T @ enT chunk via
   PE accumulation over K=512; per tile, VectorE max/max_index extract the
   top-8 values + indices of each 512-candidate chunk (global top-10 of a
   row is contained in the union of its per-chunk top-8s unless >=9 of the
   top-10 fall in one 512-chunk: P ~ 1e-11).
 - Host: merge 8 cores x 25 chunks x top-8 = 1600 candidates/query, exact
   top-10 by (value desc, index asc) = jax.lax.top_k tie order, then the
   reference's mode computation.
"""
import sys

for _p in ("/opt/trn_rl_repo", "/root/.axon_site/_ro/trn_rl_repo"):
    if _p not in sys.path:
        sys.path.insert(0, _p)

import numpy as np

import concourse.bacc as bacc
import concourse.mybir as mybir
import concourse.tile as tile
from concourse import bass_utils

F32 = mybir.dt.float32
F32R = mybir.dt.float32r
F16 = mybir.dt.float16
U32 = mybir.dt.uint32
Copy = mybir.ActivationFunctionType.Copy

B = 2048            # queries
D = 512             # embedding dim
N_EMB = 100000      # database size
K_NEIGH = 10
NUM_CLASSES = 1000
EPS = 1e-8

CORES = 8
N_PAD = 102400      # padded database size (8 * 12800)
N_CORE = N_PAD // CORES     # 12800 candidates per core
CHUNK = 512                 # candidates per sim tile (one PSUM bank)
NCHUNK = N_CORE // CHUNK    # 25
QT = B // 128               # 16 query tiles
KT = D // 128               # 4 k-tiles
NOUT = NCHUNK * 8           # 200 output slots per query per core

MM_DTYPE = "f32"            # "f32" | "f32r" | "f16x3"

_CACHE = {}


def _build(variant):
    """Build + compile the per-core Bass program. Same program on all cores;
    only the `ent*` input shards differ."""
    nc = bacc.Bacc("TRN2", target_bir_lowering=False, debug=False)

    if variant == "noop":  # minimal program for RPC-overhead baselining
        d_nin = nc.dram_tensor("nin", [128, 128], F32, kind="ExternalInput")
        d_nout = nc.dram_tensor("nout", [128, 128], F32, kind="ExternalOutput")
        with tile.TileContext(nc) as tc:
            with tc.tile_pool(name="np0", bufs=1) as pool:
                t = pool.tile([128, 128], F32, tag="t")
                nc.sync.dma_start(t[:, :], d_nin[:, :])
                nc.sync.dma_start(d_nout[:, :], t[:, :])
        nc.compile()
        return nc

    f16 = variant == "f16x3"
    if f16:
        d_xhi = nc.dram_tensor("xhi", [D, B], F16, kind="ExternalInput")
        d_xlo = nc.dram_tensor("xlo", [D, B], F16, kind="ExternalInput")
        d_ehi = nc.dram_tensor("ehi", [D, N_CORE], F16, kind="ExternalInput")
        d_elo = nc.dram_tensor("elo", [D, N_CORE], F16, kind="ExternalInput")
    else:
        in_dt = F32R if variant == "f32r" else F32
        d_xt = nc.dram_tensor("xt", [D, B], in_dt, kind="ExternalInput")
        d_ent = nc.dram_tensor("ent", [D, N_CORE], in_dt, kind="ExternalInput")

    d_vals = nc.dram_tensor("vals", [B, NOUT], F32, kind="ExternalOutput")
    d_idx = nc.dram_tensor("idx", [B, NOUT], U32, kind="ExternalOutput")

    with tile.TileContext(nc) as tc:
        with (
            tc.tile_pool(name="xpool", bufs=1) as xpool,
            tc.tile_pool(name="epool", bufs=3) as epool,
            tc.tile_pool(name="ps", bufs=6, space="PSUM") as ps_pool,
            tc.tile_pool(name="sim", bufs=6) as sim_pool,
            tc.tile_pool(name="acc", bufs=1) as acc_pool,
        ):
            # resident x (stationary operand), k-tiles side by side
            if f16:
                xhi_sb = xpool.tile([128, KT * B], F16, tag="xhi")
                xlo_sb = xpool.tile([128, KT * B], F16, tag="xlo")
                for k in range(KT):
                    nc.sync.dma_start(xhi_sb[:, k * B:(k + 1) * B],
                                      d_xhi[k * 128:(k + 1) * 128, :])
                    nc.sync.dma_start(xlo_sb[:, k * B:(k + 1) * B],
                                      d_xlo[k * 128:(k + 1) * 128, :])
            else:
                xt_sb = xpool.tile([128, KT * B], in_dt, tag="xt")
                for k in range(KT):
                    nc.sync.dma_start(xt_sb[:, k * B:(k + 1) * B],
                                      d_xt[k * 128:(k + 1) * 128, :])

            # result accumulators, [128, QT*NOUT], column q*NOUT + c*8 + j
            vals_sb = acc_pool.tile([128, QT * NOUT], F32, tag="vacc")
            idx_sb = acc_pool.tile([128, QT * NOUT], U32, tag="iacc")

            for c in range(NCHUNK):
                c0 = c * CHUNK
                if f16:
                    ehi_sb = epool.tile([128, KT * CHUNK], F16, tag="ehi")
                    elo_sb = epool.tile([128, KT * CHUNK], F16, tag="elo")
                    for k in range(KT):
                        nc.sync.dma_start(ehi_sb[:, k * CHUNK:(k + 1) * CHUNK],
                                          d_ehi[k * 128:(k + 1) * 128, c0:c0 + CHUNK])
                        nc.sync.dma_start(elo_sb[:, k * CHUNK:(k + 1) * CHUNK],
                                          d_elo[k * 128:(k + 1) * 128, c0:c0 + CHUNK])
                else:
                    en_sb = epool.tile([128, KT * CHUNK], in_dt, tag="en")
                    for k in range(KT):
                        nc.sync.dma_start(en_sb[:, k * CHUNK:(k + 1) * CHUNK],
                                          d_ent[k * 128:(k + 1) * 128, c0:c0 + CHUNK])

                for q in range(QT):
                    ps = ps_pool.tile([128, CHUNK], F32, tag="ps")
                    if variant == "f16x3":
                        nmm = 3 * KT
                        i = 0
                        for k in range(KT):
                            xh = xhi_sb[:, k * B + q * 128: k * B + (q + 1) * 128]
                            xl = xlo_sb[:, k * B + q * 128: k * B + (q + 1) * 128]
                            eh = ehi_sb[:, k * CHUNK:(k + 1) * CHUNK]
                            el = elo_sb[:, k * CHUNK:(k + 1) * CHUNK]
                            for (a, bb) in ((xh, eh), (xh, el), (xl, eh)):
                                nc.tensor.matmul(ps[:, :], a, bb,
                                                 start=(i == 0), stop=(i == nmm - 1))
                                i += 1
                    else:
                        for k in range(KT):
                            lhsT = xt_sb[:, k * B + q * 128: k * B + (q + 1) * 128]
                            rhs = en_sb[:, k * CHUNK:(k + 1) * CHUNK]
                            nc.tensor.matmul(ps[:, :], lhsT, rhs,
                                             start=(k == 0), stop=(k == KT - 1))

                    sim = sim_pool.tile([128, CHUNK], F32, tag="sim")
                    nc.scalar.activation(sim[:, :], ps[:, :], Copy)

                    o = q * NOUT + c * 8
                    nc.vector.max(vals_sb[:, o:o + 8], sim[:, :])
                    nc.vector.max_index(idx_sb[:, o:o + 8], vals_sb[:, o:o + 8],
                                        sim[:, :])

            for q in range(QT):
                nc.sync.dma_start(d_vals[q * 128:(q + 1) * 128, :],
                                  vals_sb[:, q * NOUT:(q + 1) * NOUT])
                nc.sync.dma_start(d_idx[q * 128:(q + 1) * 128, :],
                                  idx_sb[:, q * NOUT:(q + 1) * NOUT])

    nc.compile()
    return nc


def _get_nc(variant=None):
    variant = variant or MM_DTYPE
    if variant not in _CACHE:
        _CACHE[variant] = _build(variant)
    return _CACHE[variant]


def _prep_inputs(x, embeddings, variant):
    """Host prep: normalize embeddings, pad, transpose, shard; returns in_maps.

    Works per-core-shard to keep intermediates cache-sized."""
    x = np.asarray(x, dtype=np.float32)
    e = np.asarray(embeddings, dtype=np.float32)
    inv = (1.0 / np.maximum(np.linalg.norm(e, axis=1), EPS)).astype(np.float32)
    xt = np.ascontiguousarray(x.T)               # [D, B]

    in_maps = []
    for i in range(CORES):
        lo_r, hi_r = i * N_CORE, (i + 1) * N_CORE
        n_real = max(0, min(hi_r, N_EMB) - lo_r)
        ent = np.zeros((D, N_CORE), dtype=np.float32)
        if n_real > 0:
            sl = e[lo_r:lo_r + n_real]
            ent[:, :n_real] = sl.T * inv[lo_r:lo_r + n_real][None, :]
        if variant == "f16x3":
            ehi = ent.astype(np.float16)
            elo = (ent - ehi).astype(np.float16)
            in_maps.append({"ehi": ehi, "elo": elo})
        else:
            in_maps.append({"ent": ent})

    if variant == "f16x3":
        xhi = xt.astype(np.float16)
        xlo = (xt - xhi).astype(np.float16)
        for m in in_maps:
            m["xhi"] = xhi
            m["xlo"] = xlo
    else:
        for m in in_maps:
            m["xt"] = xt
    return in_maps


def _merge(results, labels):
    """Host merge: exact global top-10 from per-core per-chunk top-8 pools,
    then the reference's mode computation."""
    vals = np.concatenate([r["vals"] for r in results], axis=1)   # [B, 8*NOUT]
    idx8 = np.concatenate([r["idx"] for r in results], axis=1).astype(np.int64)

    col_base = (np.arange(NOUT, dtype=np.int64) // 8) * CHUNK      # chunk offset
    core_base = np.repeat(np.arange(CORES, dtype=np.int64) * N_CORE, NOUT)
    g = idx8 + np.tile(col_base, CORES)[None, :] + core_base[None, :]

    # padding rows (g >= N_EMB) are zero embeddings: exclude
    u = vals.view(np.uint32)
    key = np.where(u & 0x80000000, ~u, u | 0x80000000).astype(np.uint64)
    combo = ((np.uint64(0xFFFFFFFF) - key) << np.uint64(17)) | g.astype(np.uint64)
    combo[g >= N_EMB] = np.uint64(0xFFFFFFFFFFFFFFFF)
    order = np.argsort(combo, axis=1, kind="stable")[:, :K_NEIGH]
    neighbors = np.take_along_axis(g, order, axis=1)               # [B, 10]

    labels = np.asarray(labels)
    nl = labels[neighbors].astype(np.int64)                        # [B, 10]
    eq = nl[:, :, None] == nl[:, None, :]
    counts = eq.sum(-1)
    mkey = counts * (NUM_CLASSES + 1) + (NUM_CLASSES - nl)
    mi = np.argmax(mkey, axis=1)
    pred = np.take_along_axis(nl, mi[:, None], axis=1)[:, 0]
    return pred.astype(labels.dtype)


class _Runner:
    """Caches the shard_map-jitted executable across calls (mirrors
    bass2jax.run_bass_via_pjrt's multi-core path, which re-traces per call)."""

    def __init__(self, variant):
        import jax
        import concourse.mybir as mb
        from concourse import bass2jax
        from jax.experimental.shard_map import shard_map
        from jax.sharding import Mesh, PartitionSpec

        bass2jax.install_neuronx_cc_hook()
        self.jax = jax
        nc = _get_nc(variant)
        partition_name = (nc.partition_id_tensor.name
                          if nc.partition_id_tensor else None)
        in_names, out_names, out_avals, zeros = [], [], [], []
        for alloc in nc.m.functions[0].allocations:
            if not isinstance(alloc, mb.MemoryLocationSet):
                continue
            name = alloc.memorylocations[0].name
            if alloc.kind == "ExternalInput":
                if name != partition_name:
                    in_names.append(name)
            elif alloc.kind == "ExternalOutput":
                shape = tuple(alloc.tensor_shape)
                dtype = mb.dt.np(alloc.dtype)
                out_avals.append(jax.core.ShapedArray(shape, dtype))
                out_names.append(name)
                zeros.append(np.zeros((CORES * shape[0],) + shape[1:], dtype))
        self.in_names = list(in_names)
        self.out_names = out_names
        self.out_avals = out_avals
        self.zeros = zeros
        n_params = len(in_names)
        all_names = in_names + out_names
        if partition_name is not None:
            all_names = all_names + [partition_name]
        donate = tuple(range(n_params, n_params + len(out_names)))

        def _body(*args):
            operands = list(args)
            if partition_name is not None:
                operands.append(bass2jax.partition_id_tensor())
            outs = bass2jax._bass_exec_p.bind(
                *operands,
                out_avals=tuple(out_avals),
                in_names=tuple(all_names),
                out_names=tuple(out_names),
                lowering_input_output_aliases=(),
                sim_require_finite=True,
                sim_require_nnan=True,
                nc=nc,
            )
            return tuple(outs)

        devices = jax.devices()[:CORES]
        self.mesh = Mesh(np.asarray(devices), ("core",))
        self.pspec = PartitionSpec("core")
        in_specs = (self.pspec,) * (n_params + len(out_names))
        out_specs = (self.pspec,) * len(out_names)
        self.sharded = jax.jit(
            shard_map(_body, mesh=self.mesh, in_specs=in_specs,
                      out_specs=out_specs, check_rep=False),
            donate_argnums=donate, keep_unused=True,
        )

    def concat_inputs(self, in_maps):
        return [
            np.concatenate([np.asarray(m[name]) for m in in_maps], axis=0)
            for name in self.in_names
        ]

    def device_put(self, concat_in):
        from jax.sharding import NamedSharding
        sh = NamedSharding(self.mesh, self.pspec)
        return [self.jax.device_put(a, sh) for a in concat_in]

    def execute(self, concat_in):
        zeros = [np.zeros_like(z) for z in self.zeros]
        out_arrs = self.sharded(*concat_in, *zeros)
        return out_arrs

    def run(self, in_maps):
        out_arrs = self.execute(self.concat_inputs(in_maps))
        return [
            {
                name: np.asarray(out_arrs[i]).reshape(
                    CORES, *self.out_avals[i].shape)[c]
                for i, name in enumerate(self.out_names)
            }
            for c in range(CORES)
        ]


_RUNNERS = {}


def _get_runner(variant=None):
    variant = variant or MM_DTYPE
    if variant not in _RUNNERS:
        _RUNNERS[variant] = _Runner(variant)
    return _RUNNERS[variant]


def run_on_hw(x, embeddings, variant=None):
    runner = _get_runner(variant)
    in_maps = _prep_inputs(x, embeddings, variant or MM_DTYPE)
    return runner.run(in_maps)


def kernel(x, embeddings, labels):
    results = run_on_hw(x, embeddings)
    return _merge(results, labels)


# revision 16
# speedup vs baseline: 112.1365x; 112.1365x over previous
"""Trainium2 Bass kernel for BaselineKNNModel (cosine-sim KNN classifier).

Contract: kernel(**inputs) takes FULL inputs (x [2048,512] f32,
embeddings [100000,512] f32, labels [100000] int) and returns the FULL
output (pred [2048] labels.dtype), distributing work across 8 NeuronCores.

Strategy (database-parallel, per sharding hint):
 - Host: normalize embeddings (cosine denominator), pad N 100000->102400,
   transpose to [512, N]; shard along N across 8 cores (12800 each).
   x normalization is skipped: per-query positive scaling cannot change
   that query's top-k ranking.
 - Device (SPMD, per core): sim tile [128 q, 512 c] = xT.T @ enT chunk via
   PE accumulation over K=512; per tile, VectorE max/max_index extract the
   top-8 values + indices of each 512-candidate chunk (global top-10 of a
   row is contained in the union of its per-chunk top-8s unless >=9 of the
   top-10 fall in one 512-chunk: P ~ 1e-11).
 - Host: merge 8 cores x 25 chunks x top-8 = 1600 candidates/query, exact
   top-10 by (value desc, index asc) = jax.lax.top_k tie order, then the
   reference's mode computation.
"""
import sys

for _p in ("/opt/trn_rl_repo", "/root/.axon_site/_ro/trn_rl_repo"):
    if _p not in sys.path:
        sys.path.insert(0, _p)

import numpy as np

import concourse.bacc as bacc
import concourse.mybir as mybir
import concourse.tile as tile
from concourse import bass_utils

F32 = mybir.dt.float32
F32R = mybir.dt.float32r
F16 = mybir.dt.float16
U32 = mybir.dt.uint32
Copy = mybir.ActivationFunctionType.Copy

B = 2048            # queries
D = 512             # embedding dim
N_EMB = 100000      # database size
K_NEIGH = 10
NUM_CLASSES = 1000
EPS = 1e-8

CORES = 8
N_PAD = 102400      # padded database size (8 * 12800)
N_CORE = N_PAD // CORES     # 12800 candidates per core
CHUNK = 512                 # candidates per sim tile (one PSUM bank)
NCHUNK = N_CORE // CHUNK    # 25
QT = B // 128               # 16 query tiles
KT = D // 128               # 4 k-tiles
NOUT = NCHUNK * 8           # 200 output slots per query per core

# f16w variant: window-max + device window top-16 + host exact rescore
WWIN = 16                   # candidates per window
WPC = N_CORE // WWIN        # 800 windows per core
BIGCHUNK = 1024             # candidates per PSUM tile (2 banks)
NSEL = 16                   # windows kept per (query, core)
MARGIN = 4e-3               # fp16-sim error margin on unit-normalized sims
                            # (measured max |fp16 sim err| ~6e-5, ~60x safety)

MM_DTYPE = "f16w"           # "f32" | "f32r" | "f16x3" | "f16w"

_CACHE = {}


def _build(variant):
    """Build + compile the per-core Bass program. Same program on all cores;
    only the `ent*` input shards differ."""
    nc = bacc.Bacc("TRN2", target_bir_lowering=False, debug=False)

    if variant == "noop":  # minimal program for RPC-overhead baselining
        d_nin = nc.dram_tensor("nin", [128, 128], F32, kind="ExternalInput")
        d_nout = nc.dram_tensor("nout", [128, 128], F32, kind="ExternalOutput")
        with tile.TileContext(nc) as tc:
            with tc.tile_pool(name="np0", bufs=1) as pool:
                t = pool.tile([128, 128], F32, tag="t")
                nc.sync.dma_start(t[:, :], d_nin[:, :])
                nc.sync.dma_start(d_nout[:, :], t[:, :])
        nc.compile()
        return nc

    if variant == "f16w":
        return _build_f16w(nc)

    f16 = variant == "f16x3"
    if f16:
        d_xhi = nc.dram_tensor("xhi", [D, B], F16, kind="ExternalInput")
        d_xlo = nc.dram_tensor("xlo", [D, B], F16, kind="ExternalInput")
        d_ehi = nc.dram_tensor("ehi", [D, N_CORE], F16, kind="ExternalInput")
        d_elo = nc.dram_tensor("elo", [D, N_CORE], F16, kind="ExternalInput")
    else:
        in_dt = F32R if variant == "f32r" else F32
        d_xt = nc.dram_tensor("xt", [D, B], in_dt, kind="ExternalInput")
        d_ent = nc.dram_tensor("ent", [D, N_CORE], in_dt, kind="ExternalInput")

    d_vals = nc.dram_tensor("vals", [B, NOUT], F32, kind="ExternalOutput")
    d_idx = nc.dram_tensor("idx", [B, NOUT], U32, kind="ExternalOutput")

    with tile.TileContext(nc) as tc:
        with (
            tc.tile_pool(name="xpool", bufs=1) as xpool,
            tc.tile_pool(name="epool", bufs=3) as epool,
            tc.tile_pool(name="ps", bufs=6, space="PSUM") as ps_pool,
            tc.tile_pool(name="sim", bufs=6) as sim_pool,
            tc.tile_pool(name="acc", bufs=1) as acc_pool,
        ):
            # resident x (stationary operand), k-tiles side by side
            if f16:
                xhi_sb = xpool.tile([128, KT * B], F16, tag="xhi")
                xlo_sb = xpool.tile([128, KT * B], F16, tag="xlo")
                for k in range(KT):
                    nc.sync.dma_start(xhi_sb[:, k * B:(k + 1) * B],
                                      d_xhi[k * 128:(k + 1) * 128, :])
                    nc.sync.dma_start(xlo_sb[:, k * B:(k + 1) * B],
                                      d_xlo[k * 128:(k + 1) * 128, :])
            else:
                xt_sb = xpool.tile([128, KT * B], in_dt, tag="xt")
                for k in range(KT):
                    nc.sync.dma_start(xt_sb[:, k * B:(k + 1) * B],
                                      d_xt[k * 128:(k + 1) * 128, :])

            # result accumulators, [128, QT*NOUT], column q*NOUT + c*8 + j
            vals_sb = acc_pool.tile([128, QT * NOUT], F32, tag="vacc")
            idx_sb = acc_pool.tile([128, QT * NOUT], U32, tag="iacc")

            for c in range(NCHUNK):
                c0 = c * CHUNK
                if f16:
                    ehi_sb = epool.tile([128, KT * CHUNK], F16, tag="ehi")
                    elo_sb = epool.tile([128, KT * CHUNK], F16, tag="elo")
                    for k in range(KT):
                        nc.sync.dma_start(ehi_sb[:, k * CHUNK:(k + 1) * CHUNK],
                                          d_ehi[k * 128:(k + 1) * 128, c0:c0 + CHUNK])
                        nc.sync.dma_start(elo_sb[:, k * CHUNK:(k + 1) * CHUNK],
                                          d_elo[k * 128:(k + 1) * 128, c0:c0 + CHUNK])
                else:
                    en_sb = epool.tile([128, KT * CHUNK], in_dt, tag="en")
                    for k in range(KT):
                        nc.sync.dma_start(en_sb[:, k * CHUNK:(k + 1) * CHUNK],
                                          d_ent[k * 128:(k + 1) * 128, c0:c0 + CHUNK])

                for q in range(QT):
                    ps = ps_pool.tile([128, CHUNK], F32, tag="ps")
                    if variant == "f16x3":
                        nmm = 3 * KT
                        i = 0
                        for k in range(KT):
                            xh = xhi_sb[:, k * B + q * 128: k * B + (q + 1) * 128]
                            xl = xlo_sb[:, k * B + q * 128: k * B + (q + 1) * 128]
                            eh = ehi_sb[:, k * CHUNK:(k + 1) * CHUNK]
                            el = elo_sb[:, k * CHUNK:(k + 1) * CHUNK]
                            for (a, bb) in ((xh, eh), (xh, el), (xl, eh)):
                                nc.tensor.matmul(ps[:, :], a, bb,
                                                 start=(i == 0), stop=(i == nmm - 1))
                                i += 1
                    else:
                        for k in range(KT):
                            lhsT = xt_sb[:, k * B + q * 128: k * B + (q + 1) * 128]
                            rhs = en_sb[:, k * CHUNK:(k + 1) * CHUNK]
                            nc.tensor.matmul(ps[:, :], lhsT, rhs,
                                             start=(k == 0), stop=(k == KT - 1))

                    sim = sim_pool.tile([128, CHUNK], F32, tag="sim")
                    nc.scalar.activation(sim[:, :], ps[:, :], Copy)

                    o = q * NOUT + c * 8
                    nc.vector.max(vals_sb[:, o:o + 8], sim[:, :])
                    nc.vector.max_index(idx_sb[:, o:o + 8], vals_sb[:, o:o + 8],
                                        sim[:, :])

            for q in range(QT):
                nc.sync.dma_start(d_vals[q * 128:(q + 1) * 128, :],
                                  vals_sb[:, q * NOUT:(q + 1) * NOUT])
                nc.sync.dma_start(d_idx[q * 128:(q + 1) * 128, :],
                                  idx_sb[:, q * NOUT:(q + 1) * NOUT])

    nc.compile()
    return nc


def _build_f16w(nc):
    """fp16 single-pass matmul; per-tile 16-wide window max (DVE reduce,
    PSUM-direct); per-core top-16 windows per query via max/match_replace;
    host rescores the selected windows exactly."""
    Max = mybir.AluOpType.max
    X = mybir.AxisListType.X

    d_xh = nc.dram_tensor("xh", [D, B], F16, kind="ExternalInput")
    d_eh = nc.dram_tensor("eh", [D, N_CORE], F16, kind="ExternalInput")
    d_wvals = nc.dram_tensor("wvals", [B, NSEL], F32, kind="ExternalOutput")
    d_widx = nc.dram_tensor("widx", [B, NSEL], U32, kind="ExternalOutput")

    # chunk layout: 12 x 1024 + 1 x 512 = 12800
    chunks = [(i * BIGCHUNK, BIGCHUNK) for i in range(N_CORE // BIGCHUNK)]
    rem = N_CORE - (N_CORE // BIGCHUNK) * BIGCHUNK
    if rem:
        chunks.append((N_CORE - rem, rem))

    with tile.TileContext(nc) as tc:
        with (
            tc.tile_pool(name="xpool", bufs=1) as xpool,
            tc.tile_pool(name="epool", bufs=3) as epool,
            tc.tile_pool(name="ps", bufs=3, space="PSUM") as ps_pool,
            tc.tile_pool(name="wacc", bufs=1) as wacc_pool,
            tc.tile_pool(name="mrp", bufs=4) as mr_pool,
            tc.tile_pool(name="outp", bufs=1) as out_pool,
        ):
            xh_sb = xpool.tile([128, KT * B], F16, tag="xh")
            for k in range(KT):
                nc.sync.dma_start(xh_sb[:, k * B:(k + 1) * B],
                                  d_xh[k * 128:(k + 1) * 128, :])

            wmax_sb = wacc_pool.tile([128, QT * WPC], F32, tag="wacc")
            vout_sb = out_pool.tile([128, QT * NSEL], F32, tag="vout")
            iout_sb = out_pool.tile([128, QT * NSEL], U32, tag="iout")

            for (c0, cw) in chunks:
                eh_sb = epool.tile([128, KT * BIGCHUNK], F16, tag="eh")
                for k in range(KT):
                    nc.sync.dma_start(eh_sb[:, k * cw:(k + 1) * cw],
                                      d_eh[k * 128:(k + 1) * 128, c0:c0 + cw])
                for q in range(QT):
                    ps = ps_pool.tile([128, BIGCHUNK], F32, tag="ps")
                    nslice = cw // 512
                    i = 0
                    for s in range(nslice):
                        for k in range(KT):
                            nc.tensor.matmul(
                                ps[:, s * 512:(s + 1) * 512],
                                xh_sb[:, k * B + q * 128: k * B + (q + 1) * 128],
                                eh_sb[:, k * cw + s * 512: k * cw + s * 512 + 512],
                                start=(k == 0), stop=(k == KT - 1))
                            i += 1
                    nwin = cw // WWIN
                    wslot = q * WPC + c0 // WWIN
                    nc.vector.tensor_reduce(
                        wmax_sb[:, wslot:wslot + nwin],
                        ps[:, :cw].rearrange("p (w i) -> p w i", i=WWIN),
                        axis=X, op=Max)

            for q in range(QT):
                wq = wmax_sb[:, q * WPC:(q + 1) * WPC]
                o = q * NSEL
                nc.vector.max(vout_sb[:, o:o + 8], wq)
                nc.vector.max_index(iout_sb[:, o:o + 8], vout_sb[:, o:o + 8], wq)
                mr = mr_pool.tile([128, WPC], F32, tag="mr")
                nc.vector.match_replace(mr[:, :], vout_sb[:, o:o + 8], wq, -1e30)
                nc.vector.max(vout_sb[:, o + 8:o + 16], mr[:, :])
                nc.vector.max_index(iout_sb[:, o + 8:o + 16],
                                    vout_sb[:, o + 8:o + 16], mr[:, :])

            for q in range(QT):
                nc.sync.dma_start(d_wvals[q * 128:(q + 1) * 128, :],
                                  vout_sb[:, q * NSEL:(q + 1) * NSEL])
                nc.sync.dma_start(d_widx[q * 128:(q + 1) * 128, :],
                                  iout_sb[:, q * NSEL:(q + 1) * NSEL])

    nc.compile()
    return nc


def _get_nc(variant=None):
    variant = variant or MM_DTYPE
    if variant not in _CACHE:
        _CACHE[variant] = _build(variant)
    return _CACHE[variant]


def _normalize(x, embeddings):
    x = np.asarray(x, dtype=np.float32)
    e = np.asarray(embeddings, dtype=np.float32)
    xn = x / np.maximum(np.linalg.norm(x, axis=1, keepdims=True), EPS)
    inv = (1.0 / np.maximum(np.linalg.norm(e, axis=1), EPS)).astype(np.float32)
    return xn, e, inv


def _prep_f16w(xn, e, inv):
    """in_maps for the f16w variant: fp16 transposed normalized shards."""
    xh = np.ascontiguousarray(xn.T).astype(np.float16)
    in_maps = []
    for i in range(CORES):
        lo_r, hi_r = i * N_CORE, (i + 1) * N_CORE
        n_real = max(0, min(hi_r, N_EMB) - lo_r)
        eh = np.zeros((D, N_CORE), dtype=np.float16)
        if n_real > 0:
            sl = e[lo_r:lo_r + n_real] * inv[lo_r:lo_r + n_real][:, None]
            eh[:, :n_real] = sl.T.astype(np.float16)
        in_maps.append({"xh": xh, "eh": eh})
    return in_maps


def _prep_inputs(x, embeddings, variant):
    """Host prep: normalize embeddings, pad, transpose, shard; returns in_maps.

    Works per-core-shard to keep intermediates cache-sized."""
    if variant == "f16w":
        xn, e, inv = _normalize(x, embeddings)
        return _prep_f16w(xn, e, inv)
    x = np.asarray(x, dtype=np.float32)
    e = np.asarray(embeddings, dtype=np.float32)
    inv = (1.0 / np.maximum(np.linalg.norm(e, axis=1), EPS)).astype(np.float32)
    xt = np.ascontiguousarray(x.T)               # [D, B]

    in_maps = []
    for i in range(CORES):
        lo_r, hi_r = i * N_CORE, (i + 1) * N_CORE
        n_real = max(0, min(hi_r, N_EMB) - lo_r)
        ent = np.zeros((D, N_CORE), dtype=np.float32)
        if n_real > 0:
            sl = e[lo_r:lo_r + n_real]
            ent[:, :n_real] = sl.T * inv[lo_r:lo_r + n_real][None, :]
        if variant == "f16x3":
            ehi = ent.astype(np.float16)
            elo = (ent - ehi).astype(np.float16)
            in_maps.append({"ehi": ehi, "elo": elo})
        else:
            in_maps.append({"ent": ent})

    if variant == "f16x3":
        xhi = xt.astype(np.float16)
        xlo = (xt - xhi).astype(np.float16)
        for m in in_maps:
            m["xhi"] = xhi
            m["xlo"] = xlo
    else:
        for m in in_maps:
            m["xt"] = xt
    return in_maps


def _merge(results, labels):
    """Host merge: exact global top-10 from per-core per-chunk top-8 pools,
    then the reference's mode computation."""
    vals = np.concatenate([r["vals"] for r in results], axis=1)   # [B, 8*NOUT]
    idx8 = np.concatenate([r["idx"] for r in results], axis=1).astype(np.int64)

    col_base = (np.arange(NOUT, dtype=np.int64) // 8) * CHUNK      # chunk offset
    core_base = np.repeat(np.arange(CORES, dtype=np.int64) * N_CORE, NOUT)
    g = idx8 + np.tile(col_base, CORES)[None, :] + core_base[None, :]

    # padding rows (g >= N_EMB) are zero embeddings: exclude
    u = vals.view(np.uint32)
    key = np.where(u & 0x80000000, ~u, u | 0x80000000).astype(np.uint64)
    combo = ((np.uint64(0xFFFFFFFF) - key) << np.uint64(17)) | g.astype(np.uint64)
    combo[g >= N_EMB] = np.uint64(0xFFFFFFFFFFFFFFFF)
    order = np.argsort(combo, axis=1, kind="stable")[:, :K_NEIGH]
    neighbors = np.take_along_axis(g, order, axis=1)               # [B, 10]

    labels = np.asarray(labels)
    nl = labels[neighbors].astype(np.int64)                        # [B, 10]
    eq = nl[:, :, None] == nl[:, None, :]
    counts = eq.sum(-1)
    mkey = counts * (NUM_CLASSES + 1) + (NUM_CLASSES - nl)
    mi = np.argmax(mkey, axis=1)
    pred = np.take_along_axis(nl, mi[:, None], axis=1)[:, 0]
    return pred.astype(labels.dtype)


class _Runner:
    """Caches the shard_map-jitted executable across calls (mirrors
    bass2jax.run_bass_via_pjrt's multi-core path, which re-traces per call)."""

    def __init__(self, variant):
        import jax
        import concourse.mybir as mb
        from concourse import bass2jax
        from jax.experimental.shard_map import shard_map
        from jax.sharding import Mesh, PartitionSpec

        bass2jax.install_neuronx_cc_hook()
        self.jax = jax
        nc = _get_nc(variant)
        partition_name = (nc.partition_id_tensor.name
                          if nc.partition_id_tensor else None)
        in_names, out_names, out_avals, zeros = [], [], [], []
        for alloc in nc.m.functions[0].allocations:
            if not isinstance(alloc, mb.MemoryLocationSet):
                continue
            name = alloc.memorylocations[0].name
            if alloc.kind == "ExternalInput":
                if name != partition_name:
                    in_names.append(name)
            elif alloc.kind == "ExternalOutput":
                shape = tuple(alloc.tensor_shape)
                dtype = mb.dt.np(alloc.dtype)
                out_avals.append(jax.core.ShapedArray(shape, dtype))
                out_names.append(name)
                zeros.append(np.zeros((CORES * shape[0],) + shape[1:], dtype))
        self.in_names = list(in_names)
        self.out_names = out_names
        self.out_avals = out_avals
        self.zeros = zeros
        n_params = len(in_names)
        all_names = in_names + out_names
        if partition_name is not None:
            all_names = all_names + [partition_name]
        donate = tuple(range(n_params, n_params + len(out_names)))

        def _body(*args):
            operands = list(args)
            if partition_name is not None:
                operands.append(bass2jax.partition_id_tensor())
            outs = bass2jax._bass_exec_p.bind(
                *operands,
                out_avals=tuple(out_avals),
                in_names=tuple(all_names),
                out_names=tuple(out_names),
                lowering_input_output_aliases=(),
                sim_require_finite=True,
                sim_require_nnan=True,
                nc=nc,
            )
            return tuple(outs)

        devices = jax.devices()[:CORES]
        self.mesh = Mesh(np.asarray(devices), ("core",))
        self.pspec = PartitionSpec("core")
        in_specs = (self.pspec,) * (n_params + len(out_names))
        out_specs = (self.pspec,) * len(out_names)
        self.sharded = jax.jit(
            shard_map(_body, mesh=self.mesh, in_specs=in_specs,
                      out_specs=out_specs, check_rep=False),
            donate_argnums=donate, keep_unused=True,
        )

    def concat_inputs(self, in_maps):
        return [
            np.concatenate([np.asarray(m[name]) for m in in_maps], axis=0)
            for name in self.in_names
        ]

    def device_put(self, concat_in):
        from jax.sharding import NamedSharding
        sh = NamedSharding(self.mesh, self.pspec)
        return [self.jax.device_put(a, sh) for a in concat_in]

    def execute(self, concat_in):
        zeros = [np.zeros_like(z) for z in self.zeros]
        out_arrs = self.sharded(*concat_in, *zeros)
        return out_arrs

    def run(self, in_maps):
        out_arrs = self.execute(self.concat_inputs(in_maps))
        return [
            {
                name: np.asarray(out_arrs[i]).reshape(
                    CORES, *self.out_avals[i].shape)[c]
                for i, name in enumerate(self.out_names)
            }
            for c in range(CORES)
        ]


_RUNNERS = {}


def _get_runner(variant=None):
    variant = variant or MM_DTYPE
    if variant not in _RUNNERS:
        _RUNNERS[variant] = _Runner(variant)
    return _RUNNERS[variant]


def _mode_pred(neighbors, labels):
    """Reference's torch.mode semantics on gathered neighbor labels."""
    labels = np.asarray(labels)
    nl = labels[neighbors].astype(np.int64)                        # [B, 10]
    eq = nl[:, :, None] == nl[:, None, :]
    counts = eq.sum(-1)
    mkey = counts * (NUM_CLASSES + 1) + (NUM_CLASSES - nl)
    mi = np.argmax(mkey, axis=1)
    pred = np.take_along_axis(nl, mi[:, None], axis=1)[:, 0]
    return pred.astype(labels.dtype)


def _merge_f16w(results, labels, xn, e, inv):
    """Select windows >= (10th-best window max) - margin, rescore those
    candidates exactly in fp64, exact global top-10, then mode."""
    wv = np.stack([r["wvals"] for r in results], axis=1)      # [B, 8, 16]
    wi = np.stack([r["widx"] for r in results], axis=1).astype(np.int64)
    gw = wi + (np.arange(CORES, dtype=np.int64) * WPC)[None, :, None]
    wv = wv.reshape(B, CORES * NSEL)
    gw = gw.reshape(B, CORES * NSEL)

    w10 = np.partition(wv, wv.shape[1] - K_NEIGH, axis=1)[:, wv.shape[1] - K_NEIGH]
    keep = wv >= (w10[:, None] - MARGIN)
    smax = int(keep.sum(axis=1).max())

    # top-smax windows per row by value; mask out ones below the cutoff
    order = np.argsort(-wv, axis=1, kind="stable")[:, :smax]
    sel_g = np.take_along_axis(gw, order, axis=1)              # [B, smax]
    sel_keep = np.take_along_axis(keep, order, axis=1)

    # expand windows to candidate ids (global, padded domain); invalid slots
    # become an out-of-range sentinel so an ascending sort pushes them last
    cand = (sel_g[:, :, None] * WWIN +
            np.arange(WWIN, dtype=np.int64)[None, None, :]).reshape(B, -1)
    valid = np.repeat(sel_keep, WWIN, axis=1) & (cand < N_EMB)
    cand = np.where(valid, cand, np.int64(N_PAD))
    cand.sort(axis=1)  # ascending ids so stable sort on -sim ties to lower idx
    valid = cand < N_EMB

    x64 = xn.astype(np.float64)
    nsel_c = cand.shape[1]
    sims = np.empty((B, nsel_c), dtype=np.float64)
    BLK = 128
    inv64 = inv.astype(np.float64)
    e = np.asarray(e, dtype=np.float32)
    for b0 in range(0, B, BLK):
        b1 = min(b0 + BLK, B)
        g = np.minimum(cand[b0:b1], N_EMB - 1)                 # [blk, nsel_c]
        rows = e[g.reshape(-1)].astype(np.float64)
        rows *= inv64[g.reshape(-1)][:, None]
        rows = rows.reshape(b1 - b0, nsel_c, D)
        sims[b0:b1] = np.einsum("bd,bkd->bk", x64[b0:b1], rows)
    sims[~valid] = -np.inf

    # exact top-10 by (-sim, cand); cand rows are ascending so stable works
    ordr = np.argsort(-sims, axis=1, kind="stable")[:, :K_NEIGH]
    neighbors = np.take_along_axis(cand, ordr, axis=1)
    return _mode_pred(neighbors, labels)


def run_on_hw(x, embeddings, variant=None):
    runner = _get_runner(variant)
    in_maps = _prep_inputs(x, embeddings, variant or MM_DTYPE)
    return runner.run(in_maps)


def kernel(x, embeddings, labels):
    variant = MM_DTYPE
    if variant == "f16w":
        xn, e, inv = _normalize(x, embeddings)
        runner = _get_runner(variant)
        results = runner.run(_prep_f16w(xn, e, inv))
        return _merge_f16w(results, labels, xn, e, inv)
    results = run_on_hw(x, embeddings)
    return _merge(results, labels)
